# revision 45
# baseline (speedup 1.0000x reference)
"""Differential self-attention head on 8 Trainium2 NeuronCores.

Sharding: 8 cores = 4 batches x 2 softmax branches. Core c handles batch
c//2 and branch c%2 (branch 0 -> (Wq1, Wk1), branch 1 -> (Wq2, Wk2)).
Every core runs the identical SPMD program over its own data:

  - projections q,k,v with bias folded in via an augmented contraction
    (E=1024 data rows + 1 ones-row + pad to 1152 = 9 chunks of 128)
  - causal scores computed transposed [k, q] so exp(S) is directly the
    moving operand of the v^T @ p matmul (no on-chip transpose of p)
  - exp on ScalarE straight from PSUM with scale=1/sqrt(D) and a
    bias of -ln(64) (a pure rescale that cancels in num/den, keeping
    the fp16 running denominator far from overflow)
  - denominator via fp16 DVE accumulation of the exp tiles plus ONE
    ones-vector matmul per query block; the kernel's final pair is
    folded straight into that matmul so no DVE add sits on the
    end-of-kernel chain
  - v transposed into [s, D] via the XBAR DMA-transpose engine (not PE)
  - projections of block k+1 are software-pipelined into the attention
    pair slots of block k; num/mask/den for pair i run TWO pair slots
    later so the PE never waits on the exp of the pair it just scored
    (ScalarE saturates in the late blocks); v of the LAST block is
    deferred into that block's own slots as PE filler against the
    exp-bound tail
  - host prepacks x and the weights into partition-major block layouts
    so every staging DMA moves 2-8KB contiguous runs per partition
    (small descriptors capped staging at ~200GB/s); critical staging
    rides one FIFO ring in consumption order, bulk x prefetch rides the
    gpsimd SWDGE ring gated behind a WAW hazard on the attention stream
    so it cannot race the startup staging
  - 10 dummy warmup matmuls = 4.3us of gapless PE busy flip the HAM
    clock gate to 2.4GHz before the first data stall can open a gap
  - outputs the unnormalized numerator num = v^T @ p [D, S] and the
    denominator d [1, S]; the host divides and combines the two branches
    (o = num1/d1 - lam*num2/d2) and transposes back to [S, D].

All matmul operands are fp16; accumulation is fp32 in PSUM.
"""

import sys

import numpy as np

for _p in ("/opt/trn_rl_repo",):
    if _p not in sys.path:
        sys.path.insert(0, _p)

B, S, E, D = 4, 4096, 1024, 128
EA = 1152  # augmented contraction: E + ones row, padded to 9*128
QB = 512  # query block (matmul moving free dim)
KT = 128  # key tile (partition dim of transposed scores)

_PROG_CACHE = {}
LAST_RUN = None  # BassKernelResults of the most recent kernel() call


def _build_program(s, ea, qb, kt):
    import concourse.bass as bass  # noqa: F401
    import concourse.mybir as mybir
    from concourse import bacc
    from concourse.tile import TileContext

    fp16 = mybir.dt.float16
    fp32 = mybir.dt.float32
    n_ec = ea // 128  # contraction chunks
    n_sb = s // qb  # 512-wide column blocks of the full sequence
    n_st = s // kt  # 128-row key/seq tiles
    npair = qb // kt  # diag mask variants (kt tiles per query block)

    nc = bacc.Bacc("TRN2", target_bir_lowering=False, debug=False)
    # host-prepacked, partition-major layouts: every DMA moves long
    # per-partition contiguous runs (8KB per x block, 6KB of weights),
    # instead of the 1KB/256B descriptors a [ea, s] row-major layout
    # forces — measured staging bandwidth was descriptor-bound at
    # ~200GB/s with the small descriptors
    n_sb_, n_ec_ = s // qb, ea // 128
    xP = nc.dram_tensor("xP", [128, n_sb_, n_ec_, qb], fp16, kind="ExternalInput")
    wall = nc.dram_tensor("wall", [128, n_ec_, 3 * D], fp16, kind="ExternalInput")
    dmask = nc.dram_tensor("dmask", [128, kt], fp16, kind="ExternalInput")
    num_out = nc.dram_tensor("num", [D, s], fp32, kind="ExternalOutput")
    den_out = nc.dram_tensor("den", [1, s], fp32, kind="ExternalOutput")

    inv = 1.0 / np.sqrt(np.float32(D))
    exp_bias = float(-np.log(64.0))  # cancels in num/den

    with TileContext(nc) as tc:
        with (
            tc.tile_pool(name="const", bufs=1) as const_pool,
            tc.tile_pool(name="acts", bufs=1) as acts_pool,
            tc.tile_pool(name="ptiles", bufs=6) as p_pool,
            tc.tile_pool(name="outs", bufs=3) as out_pool,
            tc.tile_pool(name="accs", bufs=2) as acc_pool,
            tc.tile_pool(name="ps", bufs=2, space="PSUM") as ps_pool,
        ):
            # ---- constants ----
            # warm_src memset first: it gates warmup MM #0 on the DVE queue
            w_sb = const_pool.tile([128, n_ec, 3 * D], fp16, name="w_sb")
            warm_src = const_pool.tile([128, qb], fp16, name="warm_src")
            nc.vector.memset(warm_src, 0.0)
            ones_sb = const_pool.tile([128, 1], fp16, name="ones_sb")
            nc.vector.memset(ones_sb, 1.0)
            bias_sb = const_pool.tile([128, 1], fp32, name="bias_sb")
            nc.vector.memset(bias_sb, exp_bias)

            # block-major activation staging buffer: xt_sb[:, sb, c, :] is
            # the [128, qb] moving operand of projection chunk c of block sb,
            # and a whole block is one 8KB-per-partition contiguous DMA
            xt_sb = acts_pool.tile([128, n_sb, n_ec, qb], fp16, name="xt_sb")
            mask_sb = const_pool.tile([128, kt], fp16, name="mask_sb")

            def _dma_x(sb):
                nc.sync.dma_start(out=xt_sb[:, sb], in_=xP[:, sb])

            # critical staging all on the ONE sync ring, in consumption
            # order.  Measured: aggregate staging bandwidth does not grow
            # with more rings (a second HWDGE ring or SWDGE just splits the
            # same total), so what matters is strict priority — which
            # single-ring FIFO provides — plus chunk granularity so the
            # projection matmuls chase the transfers.
            nc.sync.dma_start(out=xt_sb[:, 0, 0:2], in_=xP[:, 0, 0:2])
            nc.sync.dma_start(out=w_sb[:, :, :], in_=wall[:, :, :])
            for c0 in range(2, n_ec, 2):
                c1 = min(c0 + 2, n_ec)
                nc.sync.dma_start(out=xt_sb[:, 0, c0:c1], in_=xP[:, 0, c0:c1])
            nc.sync.dma_start(out=mask_sb, in_=dmask[:, :])
            # x1 split in two halves: proj(1) starts consuming it almost as
            # soon as the critical set lands, and the whole-block transfer
            # finished ~1us after the first v(1) matmuls wanted chunk 0
            h = n_ec // 2
            nc.sync.dma_start(out=xt_sb[:, 1, 0:h], in_=xP[:, 1, 0:h])
            nc.sync.dma_start(out=xt_sb[:, 1, h:n_ec], in_=xP[:, 1, h:n_ec])
            nc.sync.dma_start(out=xt_sb[:, 2, 0:h], in_=xP[:, 2, 0:h])
            nc.sync.dma_start(out=xt_sb[:, 2, h:n_ec], in_=xP[:, 2, h:n_ec])

            qT = acts_pool.tile([128, s], fp16, name="qT")
            kTt = acts_pool.tile([128, s], fp16, name="kTt")
            vT = acts_pool.tile([128, s], fp16, name="vT")
            v_sb = acts_pool.tile([128, n_st, D], fp16, name="v_sb")
            den_sb = out_pool.tile([1, s], fp32, name="den_sb", bufs=1)

            # ---- PE warmup: dummy matmuls while the first DMAs land flip
            # the HAM clock gate to full rate.  The un-throttle fires only
            # after one full GAPLESS ~3.4us busy window, and its free-running
            # phase adds +-0.5us — 8 matmuls (3.39us) lost that race on some
            # runs and left the whole DMA-stalled early stream at 1.2GHz
            # (measured: cold until 20.6us).  10 matmuls = 4.3us is safely
            # past the window before the first data stall can open a gap.
            wp = ps_pool.tile([128, qb], fp32, name="wp", tag="np", bufs=2)
            for _ in range(10):
                nc.tensor.matmul(
                    wp, lhsT=warm_src[:, 0:D], rhs=warm_src, start=True, stop=True
                )

            def emit_proj(sb, parts=(2, 0, 1)):
                """Generator: each next() emits one PE op (with its attached
                DVE copy when a PSUM tile completes) of block sb's q/k/v
                projections + v transposes.  parts selects which of v(2),
                q(0), k(1) to emit, in order."""
                # v first: its consumers (the diagonal num matmuls of the
                # next attention block) sit at the end of the longest chain
                # (proj psum -> DVE copy -> XBAR DMA-transpose -> num), so
                # the transpose must be in flight as early as possible.
                # q second (needed by the next block's first score matmuls);
                # k's own-block tiles are only read late in that block.
                dsts = {0: qT, 1: kTt, 2: vT}
                for mi in parts:
                    dst = dsts[mi]
                    pps = ps_pool.tile([128, qb], fp32, name="pps", tag="pj", bufs=2)
                    for c in range(n_ec):
                        nc.tensor.matmul(
                            pps,
                            lhsT=w_sb[:, c, mi * D : (mi + 1) * D],
                            rhs=xt_sb[:, sb, c, :],
                            start=(c == 0),
                            stop=(c == n_ec - 1),
                        )
                        if c == n_ec - 1:
                            nc.vector.tensor_copy(
                                dst[:, sb * qb : (sb + 1) * qb], pps
                            )
                            if mi == 2:
                                # natural [s, D] layout via the XBAR DMA
                                # transpose: one 3D-output call per block,
                                # out[p, c, d] = in[d, c*128+p] (HW-verified)
                                # = exactly v_sb's [s-tile] layout
                                nc.sync.dma_start_transpose(
                                    v_sb[
                                        :,
                                        sb * (qb // 128) : (sb + 1) * (qb // 128),
                                        :,
                                    ],
                                    vT[:, sb * qb : (sb + 1) * qb],
                                )
                        yield

            def finish_pair(nkt, halves, pt, nump, acc, skip_den=False):
                """Mask + num-matmul + denominator accumulation for a pair
                whose exp has been emitted.  skip_den leaves the denominator
                to the caller (the last pair folds it into the dp matmul so
                the final DVE adds drop off the end-of-kernel chain)."""
                for ktile, j, qo, h in halves:
                    if j >= 0:
                        # triangular mask on the 128 columns at the diagonal
                        nc.vector.tensor_mul(
                            pt[:, h * qb + qo : h * qb + qo + kt],
                            pt[:, h * qb + qo : h * qb + qo + kt],
                            mask_sb[:, 0:kt],
                        )
                    nc.tensor.matmul(
                        nump[:, qo:qb],
                        lhsT=v_sb[:, ktile, :],
                        rhs=pt[:, h * qb + qo : (h + 1) * qb],
                        start=(ktile == 0),
                        stop=(ktile == nkt - 1),
                    )
                    # all den accumulation on DVE: a DVE tensor_tensor and any
                    # GpSimd op arbitrate for the same exclusive SBUF port
                    # pair, so splitting the adds across both engines just
                    # serializes them with extra overhead (GpSimd is ~3x
                    # slower per add on top)
                    if skip_den:
                        pass
                    elif ktile == 0:
                        # qo == 0 for ktile 0 in every block
                        nc.vector.tensor_copy(acc, pt[:, 0:qb])
                    else:
                        nc.vector.tensor_add(
                            acc[:, qo:qb],
                            acc[:, qo:qb],
                            pt[:, h * qb + qo : (h + 1) * qb],
                        )

            # ---- fused projection + attention pipeline ----
            # proj block 0 up front; proj block k+1 is spread across the
            # attention pair-slots of block k.
            def emit_tail(qbi, nump, acc, fold=None):
                """Block epilogue: denominator matmuls + output staging.
                fold=(pt, halves) folds that pair's den contribution straight
                into the dp matmul (used for the kernel's final pair)."""
                qs = slice(qbi * qb, (qbi + 1) * qb)
                # denominator: one accumulating ones-vector matmul over the
                # fp16 accumulators (PE cost ~0.4us/block vs ~4us/block for
                # the per-key-tile rowsum variant)
                dp = ps_pool.tile([1, qb], fp32, name="dp", tag="pj", bufs=2)
                nc.tensor.matmul(
                    dp, lhsT=ones_sb, rhs=acc, start=True, stop=fold is None
                )
                if fold is not None:
                    pt_, halves_ = fold
                    for ktile, j, qo, h in halves_:
                        nc.tensor.matmul(
                            dp[:, qo:qb],
                            lhsT=ones_sb,
                            rhs=pt_[:, h * qb + qo : (h + 1) * qb],
                            start=False,
                            stop=(ktile == halves_[-1][0]),
                        )
                numo = out_pool.tile([128, qb], fp32, name="numo", tag="numo")
                nc.vector.tensor_copy(numo, nump)
                nc.sync.dma_start(out=num_out[:, qs], in_=numo)
                nc.vector.tensor_copy(den_sb[:, qs], dp)
                if qbi == n_sb - 1:
                    # the whole denominator goes out in one 16KB DMA at the
                    # end instead of 8 tiny per-block issues on the sync queue
                    nc.sync.dma_start(out=den_out[:, :], in_=den_sb[:, :])

            for _ in emit_proj(0):
                pass
            _prefetch_x(3, qT[:, 0:1])

            def _prefetch_x(sb, gate_src):
                """x block prefetch on the gpsimd SWDGE ring, paced by the
                attention stream: engine queues run dataflow-style, so a
                free-floating prefetch issues immediately and its 1MB
                round-robins HBM bandwidth away from the critical startup
                staging (+6..12us measured).  The 1-element copy reads an
                SBUF tile the stream produces ~3 blocks before this block's
                data is consumed, and writes into the DMA's own destination
                — the WAW hazard is an ordering edge the Tile scheduler must
                respect, and it is anchored to dataflow, not to the
                scheduler's (drifting) simulated clock."""
                nc.gpsimd.tensor_copy(
                    xt_sb[:, sb, 0:1, 0:1], gate_src
                )
                nc.gpsimd.dma_start(out=xt_sb[:, sb], in_=xP[:, sb])

            # the attention runs as one continuous pair stream; num/mask/den
            # for pair i run TWO slots later (after the scores of pairs i+1
            # and i+2), so the PE never parks on exp(p): by the time the num
            # matmul issues, its exp has had ~2 full pair-slots to complete.
            # Block tails ride on the pending entry of the block's last pair.
            pendings = []  # [(nkt, halves, pt, nump, acc, tail|None), ...]

            def finish_one():
                nkt_, halves_, pt_, nump_, acc_, tail_ = pendings.pop(0)
                last = tail_ is not None and tail_[0] == n_sb - 1
                finish_pair(nkt_, halves_, pt_, nump_, acc_, skip_den=last)
                if tail_ is not None:
                    emit_tail(*tail_, fold=(pt_, halves_) if last else None)

            for qbi in range(n_sb):
                nkt = (qbi + 1) * npair  # causal: key tiles needed
                nump = ps_pool.tile([128, qb], fp32, name="nump", tag="np", bufs=2)
                acc = acc_pool.tile([128, qb], fp16, name="acc", tag="acc")
                # projection placement: proj(b+1) normally rides block b's
                # pair slots, but the LAST block's attention is exp-bound
                # (ACT needs ~1147ns/pair vs the PE's ~864ns of score+num
                # work), so v(7) is deferred into block 7's own slots as PE
                # filler — legal because only the final diagonal pairs of a
                # block read its own v tiles.  q(7)/k(7) stay in block 6
                # (q feeds block 7's first scores; k its late scores).
                if qbi + 1 < n_sb:
                    parts = (2, 0, 1) if qbi + 1 < n_sb - 1 else (0, 1)
                    gen = emit_proj(qbi + 1, parts)
                    n_gen_ops = len(parts) * n_ec
                elif n_sb >= 2:
                    gen = emit_proj(qbi, (2,))  # v of the last block
                    n_gen_ops = n_ec
                else:
                    gen, n_gen_ops = iter(()), 0
                nslots = nkt // 2
                emitted = 0
                for ktp in range(nslots):
                    k0 = 2 * ktp
                    sp = ps_pool.tile([128, 2 * qb], fp32, name="sp", tag="sp", bufs=2)
                    pt = p_pool.tile([128, 2 * qb], fp16, name="pt", tag="pt")
                    halves = []
                    for h in range(2):
                        ktile = k0 + h
                        # diagonal tiles (j >= 0) only need q >= j*kt:
                        # skip the all-masked left part of the tile
                        j = ktile - (nkt - npair)
                        qo = max(j, 0) * kt
                        nc.tensor.matmul(
                            sp[:, h * qb + qo : (h + 1) * qb],
                            lhsT=kTt[:, ktile * kt : (ktile + 1) * kt],
                            rhs=qT[:, qbi * qb + qo : (qbi + 1) * qb],
                            start=True,
                            stop=True,
                        )
                        halves.append((ktile, j, qo, h))
                    if halves[0][2] == 0:
                        # one wide exp across both halves, even when half 1
                        # starts at qo>0: exp'ing the stale [qb, qb+qo) gap
                        # costs qo/1.2 ns but saves a second ACTIVATE's ~293ns
                        # fixed cost (net win for qo=128; the gap holds old
                        # finite scores, and its pt output is never read)
                        nc.scalar.activation(
                            pt,
                            sp,
                            mybir.ActivationFunctionType.Exp,
                            scale=float(inv),
                            bias=bias_sb,
                        )
                    else:
                        for ktile, j, qo, h in halves:
                            nc.scalar.activation(
                                pt[:, h * qb + qo : (h + 1) * qb],
                                sp[:, h * qb + qo : (h + 1) * qb],
                                mybir.ActivationFunctionType.Exp,
                                scale=float(inv),
                                bias=bias_sb,
                            )
                    if ktp == 0 and qbi + 4 < n_sb:
                        _prefetch_x(qbi + 4, pt[:, 0:1])
                    # interleaved projection ops; the last block's own-v
                    # filler is front-loaded (2 ops/slot) so its XBAR
                    # transpose completes well before the diagonal nums
                    if qbi == n_sb - 1:
                        quota = min(n_gen_ops, (ktp + 1) * 2)
                    else:
                        quota = ((ktp + 1) * n_gen_ops) // nslots
                    while emitted < quota and next(gen, 1) is None:
                        emitted += 1
                    while len(pendings) >= 2:
                        finish_one()
                    pendings.append((nkt, halves, pt, nump, acc, None))
                for _ in gen:
                    pass
                # attach this block's tail to its last pair
                nkt_, halves_, pt_, nump_, acc_, _ = pendings[-1]
                pendings[-1] = (nkt_, halves_, pt_, nump_, acc_, (qbi, nump, acc))
            while pendings:
                finish_one()
    nc.compile()
    return nc


def _prep_inputs(x, Wq1, bq1, Wq2, bq2, Wk1, bk1, Wk2, bk2, Wv, bv):
    """Host-side data prep: fp16 activations + weights, prepacked into the
    partition-major block layouts the kernel DMAs expect (long contiguous
    per-partition runs = big DMA descriptors).  When all biases are zero
    (the standard case) skip the bias-fold augmentation row and its extra
    contraction chunk."""
    biases = [np.asarray(b, dtype=np.float32) for b in (bq1, bq2, bk1, bk2, bv)]
    need_aug = any(np.any(b) for b in biases)
    ea = EA if need_aug else E
    n_ec, n_sb = ea // 128, S // QB

    x = np.asarray(x, dtype=np.float32)
    xT = np.zeros((B, ea, S), dtype=np.float16)
    xT[:, :E, :] = x.transpose(0, 2, 1).astype(np.float16)
    if need_aug:
        xT[:, E, :] = 1.0  # ones row: folds the bias into the matmul
    # [B, ea, S] -> [B, 128(part), n_sb, n_ec, QB]
    xPk = np.ascontiguousarray(
        xT.reshape(B, n_ec, 128, n_sb, QB).transpose(0, 2, 3, 1, 4)
    )

    def aug(W, b):
        Wa = np.zeros((ea, D), dtype=np.float16)
        Wa[:E] = np.asarray(W, dtype=np.float32).astype(np.float16)
        if need_aug:
            Wa[E] = np.asarray(b, dtype=np.float32).astype(np.float16)
        return Wa

    # wall[p, c, 0:D]=wq, [D:2D]=wk, [2D:3D]=wv  (matches w_sb's layout)
    def pack_wall(wq_a, wk_a, wv_a):
        wall = np.empty((128, n_ec, 3 * D), dtype=np.float16)
        for mi, wa in ((0, wq_a), (1, wk_a), (2, wv_a)):
            wall[:, :, mi * D : (mi + 1) * D] = wa.reshape(
                n_ec, 128, D
            ).transpose(1, 0, 2)
        return wall

    wv_a = aug(Wv, bv)
    walls = [
        pack_wall(aug(Wq1, bq1), aug(Wk1, bk1), wv_a),
        pack_wall(aug(Wq2, bq2), aug(Wk2, bk2), wv_a),
    ]

    # triangular 0/1 mask for the 128 columns at the causal diagonal
    ki = np.arange(KT)[:, None]
    ci = np.arange(KT)[None, :]
    dm = (ci >= ki).astype(np.float16)
    return xPk, walls, dm, ea


def kernel(x, Wq1, bq1, Wq2, bq2, Wk1, bk1, Wk2, bk2, Wv, bv, lam, mask):
    from concourse.bass_utils import run_bass_kernel_spmd

    xPk, walls, dm, ea = _prep_inputs(
        x, Wq1, bq1, Wq2, bq2, Wk1, bk1, Wk2, bk2, Wv, bv
    )

    key = (S, ea, QB, KT)
    if key not in _PROG_CACHE:
        _PROG_CACHE[key] = _build_program(*key)
    nc = _PROG_CACHE[key]

    in_maps = []
    for c in range(8):
        b, br = c // 2, c % 2
        in_maps.append(
            {
                "xP": xPk[b],
                "wall": walls[br],
                "dmask": dm,
            }
        )
    global LAST_RUN
    lam = np.float32(np.asarray(lam))
    for attempt in range(3):
        run = run_bass_kernel_spmd(nc, in_maps, core_ids=list(range(8)))
        LAST_RUN = run
        res = run.results
        out = np.empty((B, S, D), dtype=np.float32)
        for b in range(B):
            n1, d1 = res[2 * b]["num"], res[2 * b]["den"]
            n2, d2 = res[2 * b + 1]["num"], res[2 * b + 1]["den"]
            out[b] = (n1 / d1 - lam * (n2 / d2)).T
        # transient device flakes have produced non-finite garbage once in
        # ~dozens of runs; a clean re-execution has always recovered
        if np.isfinite(out).all():
            break
    return out

